# revision 47
# baseline (speedup 1.0000x reference)
"""Baichuan sliding-window GQA attention block on 8 trn2 NeuronCores.

Sharding: data-parallel over batch (2) x tensor-parallel over heads (4).
Core c handles batch b=c//4, head group g=c%4 (q heads 4g..4g+3, kv heads
2g..2g+1). Each core computes qkv projection, RoPE, 2-tap causal conv,
windowed attention and a row-sharded o_proj partial; the host sums the 4
partials per batch.

All on-chip tensors live in a transposed (feature, token) layout so the
tensor engine's contraction (partition) axis lines up without transposes:
  scoresT[k,q] = sum_d kT[d,k] qT[d,q];  outT[d,q] = sum_k v[k,d] probsT[k,q]
V alone is flipped to (token, dim) via PE transposes.

The qkv projection and o_proj run as fp8 DoubleRow matmuls (2 contraction
tiles per instruction at 0.5 cycles/row) with an error-compensated 3-term
split: x ~ x_hi + x_lo, w ~ w_hi + w_lo (each e4m3, power-of-2 pre-scaled
so the lo residual stays in normal range), and
x@w ~ x_hi@w_hi + x_lo@w_hi + x_hi@w_lo  (0.75x the bf16 cycle count).
The fixed descales fold into the rope cos/sin tables, the v-conv weights,
the softmax ones-vector and the output copy. Attention internals (scores,
probs, V) stay fp32.

Layouts are chosen so the dependency tracker's range checks stay
chunk-local: qpair is (token, head)-interleaved and attn is
(token, head)-interleaved, so attention unit (qi,i) reads only chunk-qi
byte ranges of qpair and o_proj block t4 reads only its own token range
of attn. PSUM pools are flat and shared by tag across phases (qkv chunk
tiles double as attention scores tiles, pv doubles as o_proj psum), so
attention's first allocations chain onto the earliest-freed banks of the
last qkv chunk. o_proj blocks are emitted between attention chunk-pairs
(block t4 right after qi=2*t4+1) so its DoubleRows fill attention's
softmax-latency gaps and the output DMA spreads across the phase.
"""

import numpy as np
import ml_dtypes

B, S, H = 2, 2048, 2048
NH, NKV, HD = 16, 8, 128
WINDOW = 1024
THETA = 100000.0
TP = 4                      # tensor-parallel ways (head groups)
QH = NH // TP               # 4 q heads per core
KVH = NKV // TP             # 2 kv heads per core
NCORES = 8
SCALE = 1.0 / float(np.sqrt(HD))
NEG = -1.0e30

NT = S // 256               # 8 token chunks of 256
NK = H // 128               # 16 contraction tiles
NKP = NK // 2               # 8 contraction tile-pairs (DoubleRow)

# fp8 split scales (powers of two; descale folded into constants)
A_H = 16.0                  # hidden
B_W = 256.0                 # W_pack
QKV_DESCALE = 1.0 / (A_H * B_W)
A_AT = 32.0                 # attn (max |attn| <= max|v| ~ 5.5 -> 176 < 240)
B_WO = 256.0                # W_o
O_DESCALE = 1.0 / (A_AT * B_WO)

# stage0 blob layout (f8e4 cols): 8 k-pair groups of
# [wh_pair 2048 | xh_pair 512 | wl_pair 2048 | xl_pair 512]
_GRP = 5120
_ST0 = NKP * _GRP           # 40960

_CACHE = {}


def _build_program():
    import concourse.bacc as bacc
    import concourse.mybir as mybir
    import concourse.tile as tile

    f32 = mybir.dt.float32
    f32r = mybir.dt.float32r
    bf16 = mybir.dt.bfloat16
    f8 = mybir.dt.float8e4
    DR = mybir.MatmulPerfMode.DoubleRow
    Exp = mybir.ActivationFunctionType.Exp
    mult = mybir.AluOpType.mult
    add = mybir.AluOpType.add

    nc = bacc.Bacc("TRN2", target_bir_lowering=False, debug=False,
                   enable_asserts=False, num_devices=NCORES)

    cw_d = nc.dram_tensor("cw", [128, 8], f32, kind="ExternalInput")
    oe_d = nc.dram_tensor("oe", [128, 256], f32r, kind="ExternalInput")
    pre_d = nc.dram_tensor("pre", [128, 512], bf16, kind="ExternalInput")
    st0_d = nc.dram_tensor("st0", [128, _ST0], f8, kind="ExternalInput")
    csn_d = nc.dram_tensor("csn", [128, 2 * S], bf16, kind="ExternalInput")
    hb_d = nc.dram_tensor("hb", [128, 7 * 8192], f8, kind="ExternalInput")
    wob_d = nc.dram_tensor("wob", [128, 2 * QH * 2048], f8, kind="ExternalInput")
    msk_d = nc.dram_tensor("msk", [128, 2048], f32r, kind="ExternalInput")
    yT_d = nc.dram_tensor("yT", [H, S], bf16, kind="ExternalOutput")

    with tile.TileContext(nc) as tc:
        with (
            tc.tile_pool(name="const", bufs=1) as cp,
            tc.tile_pool(name="persist", bufs=1) as pp,
        ):
            cw_sb = cp.tile([128, 8], f32, tag="cw", name="cw_sb")
            oe_sb = cp.tile([128, 256], f32r, tag="oe", name="oe_sb")
            pre_sb = cp.tile([128, 512], bf16, tag="pre", name="pre_sb")
            wo_sb = cp.tile([128, 2 * QH * 2048], f8, tag="wob", name="wo_sb")
            msk_sb = cp.tile([128, 2048], f32r, tag="msk", name="msk_sb")

            # persistent across phases; qpair is (token, head)-interleaved
            qpair = [pp.tile([128, 2 * S], f32r, tag=f"qp{i}", name=f"qp{i}") for i in range(KVH)]
            kconv = [pp.tile([128, S], f32r, tag=f"kc{i}", name=f"kc{i}") for i in range(KVH)]
            vt = [[pp.tile([128, 128], f32r, tag=f"vt{i}_{j}", name=f"vt{i}_{j}") for j in range(NK)]
                  for i in range(KVH)]

            with tc.tile_pool(name="bst", bufs=1) as bs:
                st0 = bs.tile([128, _ST0], f8, tag="st0", name="st0")
                one_sb = oe_sb[:, 0:128]        # value 1/A_AT
                eye_sb = oe_sb[:, 128:256]

                def whv(kp, c):
                    v = st0[:, kp * _GRP + 512 + c * 256:
                            kp * _GRP + 512 + c * 256 + 256]
                    return v.rearrange("p (two m) -> p two m", two=2)

                def xh0v(kp):
                    v = st0[:, kp * _GRP:kp * _GRP + 512]
                    return v.rearrange("p (two n) -> p two n", two=2)

                def wlv(kp, c):
                    v = st0[:, kp * _GRP + 3072 + c * 256:
                            kp * _GRP + 3072 + c * 256 + 256]
                    return v.rearrange("p (two m) -> p two m", two=2)

                def xl0v(kp):
                    v = st0[:, kp * _GRP + 2560:kp * _GRP + 3072]
                    return v.rearrange("p (two n) -> p two n", two=2)

                # exp-path pool allocated before the phase-B pools so its
                # space is fresh (no WAR on phase-B drains at attention start)
                with tc.tile_pool(name="awE", bufs=3) as awe:
                  with (
                    tc.tile_pool(name="bcsn", bufs=1) as bc,
                    tc.tile_pool(name="bhb", bufs=3) as bh,
                    tc.tile_pool(name="broll", bufs=1) as br,
                    tc.tile_pool(name="btmp", bufs=2) as bt,
                    tc.tile_pool(name="bps", bufs=5, space="PSUM") as psb,
                    tc.tile_pool(name="bps2", bufs=2, space="PSUM") as pse,
                    tc.tile_pool(name="bpst", bufs=1, space="PSUM") as psm,
                  ):
                    # ---- phase B: fused qkv projection + rope + conv +
                    # v-transpose ----
                    for kp in range(NKP):
                        ga = kp * _GRP
                        nsp = 2
                        w = _GRP // nsp
                        for sp in range(nsp):
                            nc.sync.dma_start(
                                out=st0[:, ga + sp * w:ga + (sp + 1) * w],
                                in_=st0_d[:, ga + sp * w:ga + (sp + 1) * w])
                        if kp == 1:
                            nc.sync.dma_start(out=pre_sb[:], in_=pre_d[:, :])
                            nc.sync.dma_start(out=cw_sb[:], in_=cw_d[:, :])
                            nc.sync.dma_start(out=oe_sb[:], in_=oe_d[:, :])
                    csn_sb = bc.tile([128, 2 * S], bf16, tag="csn", name="csn_sb")
                    hbt = []
                    for t in range(1, NT):
                        ht = bh.tile([128, 8192], f8, tag="hb", name=f"hb{t}")
                        hbt.append(ht)
                        nq = 4 if t == 1 else 2
                        for qtr in range(nq):
                            w = 8192 // nq
                            nc.sync.dma_start(
                                out=ht[:, qtr * w:(qtr + 1) * w],
                                in_=hb_d[:, (t - 1) * 8192 + qtr * w:
                                         (t - 1) * 8192 + (qtr + 1) * w])
                        if t == 2:
                            nc.sync.dma_start(out=csn_sb[:], in_=csn_d[:, :])
                            nc.sync.dma_start(out=wo_sb[:], in_=wob_d[:, :])
                            nc.sync.dma_start(out=msk_sb[:], in_=msk_d[:, :])

                    def xhv(t, kp):
                        v = hbt[t - 1][:, kp * 1024:kp * 1024 + 512]
                        return v.rearrange("p (two n) -> p two n", two=2)

                    def xlv(t, kp):
                        v = hbt[t - 1][:, kp * 1024 + 512:kp * 1024 + 1024]
                        return v.rearrange("p (two n) -> p two n", two=2)

                    kbuf = br.tile([128, 1024], f32, name="kbuf")
                    vbuf = br.tile([128, 1024], f32, name="vbuf")
                    for t in range(NT):
                        cur, prv = (t % 2) * 256, ((t + 1) % 2) * 256
                        if t == 0:
                            csl = pre_sb[:, 0:256]
                            snl = pre_sb[:, 256:512]
                        else:
                            csl = csn_sb[:, t * 256:(t + 1) * 256]
                            snl = csn_sb[:, S + t * 256:S + (t + 1) * 256]
                        if t == 0:
                            # kp-outer over all 8 cols (4 psum banks of col
                            # pairs): each arriving DMA piece unlocks a wave
                            # of DoubleRows, so PE ramps with the DMA
                            psc0 = [psb.tile([128, 512], f32, tag="qkps",
                                             name=f"qk0_{c}") for c in range(4)]
                            for kp in range(NKP):
                                for c in range(8):
                                    # start=True lazily zeroes the whole 2KB
                                    # psum bank: only the even half may carry
                                    # it; the odd half's kp=0 lands on bytes
                                    # already marked pending-zero
                                    nc.tensor.matmul(
                                        psc0[c // 2][:, (c % 2) * 256:(c % 2) * 256 + 256],
                                        whv(kp, c),
                                        xh0v(kp),
                                        start=(kp == 0 and c % 2 == 0),
                                        stop=False, perf_mode=DR,
                                        skip_group_check=True)
                                for c in range(8):
                                    nc.tensor.matmul(
                                        psc0[c // 2][:, (c % 2) * 256:(c % 2) * 256 + 256],
                                        wlv(kp, c),
                                        xh0v(kp),
                                        start=False, stop=False, perf_mode=DR,
                                        skip_group_check=True)
                                    nc.tensor.matmul(
                                        psc0[c // 2][:, (c % 2) * 256:(c % 2) * 256 + 256],
                                        whv(kp, c),
                                        xl0v(kp),
                                        start=False,
                                        stop=(kp == NKP - 1), perf_mode=DR,
                                        skip_group_check=True)
                        # emit all 8 cols' DoubleRows first, then drain with
                        # the conv chain emitted right after k/v
                        porder = (1, 0, 3, 2) if t == NT - 1 else (3, 2, 0, 1)
                        pstile = {}
                        for c4 in porder:
                            if t == 0:
                                pstile[c4] = psc0[c4]
                            else:
                                ps2 = psb.tile([128, 512], f32, tag="qkps",
                                               name="qkps")
                                pstile[c4] = ps2
                                for hh in range(2):
                                    col = 2 * c4 + hh
                                    po = ps2[:, hh * 256:(hh + 1) * 256]
                                    for kp in range(NKP):
                                        lhs_h = whv(kp, col)
                                        lhs_l = wlv(kp, col)
                                        nc.tensor.matmul(
                                            po, lhs_h, xhv(t, kp),
                                            start=(kp == 0), stop=False,
                                            perf_mode=DR, skip_group_check=True)
                                        nc.tensor.matmul(
                                            po, lhs_l, xhv(t, kp),
                                            start=False, stop=False,
                                            perf_mode=DR, skip_group_check=True)
                                        nc.tensor.matmul(
                                            po, lhs_h, xlv(t, kp),
                                            start=False, stop=(kp == NKP - 1),
                                            perf_mode=DR, skip_group_check=True)

                        csb = csl.unsqueeze(1).broadcast_to([128, 2, 256])
                        snb = snl.unsqueeze(1).broadcast_to([128, 2, 256])

                        def drain_pair(c4):
                            # both columns of the psum pair drained in single
                            # wide DVE ops: cos/sin broadcast over the column
                            # dim, outputs viewed (s, col) to match the
                            # interleaved destinations
                            ps = pstile[c4][:]
                            if c4 == 3:
                                nc.scalar.copy(vbuf[:, cur * 2:cur * 2 + 512],
                                               ps)
                                return
                            e1 = bt.tile([128, 512], f32, tag="e1", name="e1")
                            # e2 must be PSUM: SB+SB operands with different
                            # base partitions are illegal
                            e2 = pse.tile([128, 512], f32, tag="e2",
                                          name="e2")
                            ps3 = ps.rearrange("p (h s) -> p h s", h=2)
                            nc.vector.tensor_mul(
                                e1[:].rearrange("p (h s) -> p h s", h=2),
                                ps3, csb)
                            nc.vector.tensor_mul(
                                e2[:].rearrange("p (h s) -> p h s", h=2),
                                ps3, snb)
                            e1v = e1[:].rearrange("p (h s) -> p s h", h=2)
                            e2v = e2[:].rearrange("p (h s) -> p s h", h=2)
                            if c4 < 2:
                                q3 = qpair[c4][:].rearrange(
                                    "p (s h) -> p s h", h=2)
                                d0 = q3[0:64, t * 256:(t + 1) * 256, :]
                                d1 = q3[64:128, t * 256:(t + 1) * 256, :]
                            else:
                                k3 = kbuf[:].rearrange(
                                    "p (ki s) -> p s ki", ki=2)
                                d0 = k3[0:64, cur:cur + 256, :]
                                d1 = k3[64:128, cur:cur + 256, :]
                            nc.vector.tensor_sub(d0, e1v[0:64, :, :],
                                                 e2v[64:128, :, :])
                            nc.vector.tensor_add(d1, e2v[0:64, :, :],
                                                 e1v[64:128, :, :])

                        if t == NT - 1:
                            drain_pair(1)
                            drain_pair(0)
                        drain_pair(3)

                        def conv2(src_cur, src_prev1, dst, dst_off, w0c,
                                  w1c, tagp):
                            # 2-tap causal conv on Pool (TensorTensor with
                            # stride-0 broadcast weights; Pool cannot run
                            # TensorScalarPtr or touch PSUM). src_cur is the
                            # chunk's [128,256] slice; src_prev1 the previous
                            # chunk's last column.
                            w0 = cw_sb[:, w0c:w0c + 1]
                            w1 = cw_sb[:, w1c:w1c + 1]
                            tmA = bt.tile([128, 256], f32, tag=tagp + "a",
                                          name=tagp + "a")
                            tmB = bt.tile([128, 256], f32, tag=tagp + "b",
                                          name=tagp + "b")
                            nc.gpsimd.tensor_mul(tmA[:], src_cur,
                                                 w1.broadcast_to([128, 256]))
                            nc.gpsimd.tensor_mul(tmB[:, 1:256],
                                                 src_cur[:, 0:255],
                                                 w0.broadcast_to([128, 255]))
                            if t == 0:
                                nc.gpsimd.tensor_copy(dst[:, dst_off:dst_off + 1],
                                                      tmA[:, 0:1])
                            else:
                                nc.gpsimd.tensor_mul(tmB[:, 0:1], src_prev1, w0)
                                nc.gpsimd.tensor_add(dst[:, dst_off:dst_off + 1],
                                                     tmA[:, 0:1], tmB[:, 0:1])
                            nc.gpsimd.tensor_add(
                                dst[:, dst_off + 1:dst_off + 256],
                                tmA[:, 1:256], tmB[:, 1:256])

                        for i in range(KVH):
                            vcb = bt.tile([128, 256], f32r, tag="vcb", name="vcb")
                            conv2(vbuf[:, cur * 2 + i * 256:
                                       cur * 2 + i * 256 + 256],
                                  vbuf[:, prv * 2 + i * 256 + 255:
                                       prv * 2 + i * 256 + 256],
                                  vcb, 0, 4 + 2 * i, 5 + 2 * i, "cv")
                            for h in range(2):
                                tp = psm.tile([128, 128], f32r, tag="vtp", name="vtp")
                                nc.tensor.transpose(tp[:], vcb[:, h * 128:(h + 1) * 128],
                                                    eye_sb)
                                nc.scalar.copy(vt[i][2 * t + h][:], tp[:])
                        drain_pair(2)
                        for i in range(KVH):
                            conv2(kbuf[:, i * 512 + cur:i * 512 + cur + 256],
                                  kbuf[:, i * 512 + prv + 255:
                                       i * 512 + prv + 256],
                                  kconv[i], t * 256, 2 * i, 2 * i + 1, "ck")
                        if t != NT - 1:
                            drain_pair(0)
                            drain_pair(1)

                  # ---- phase E: banded attention, with o_proj (phase F)
                  # blocks interleaved after every odd chunk ----
                  with (
                        tc.tile_pool(name="att", bufs=1) as ap,
                        tc.tile_pool(name="atw", bufs=3) as aw,
                        tc.tile_pool(name="eps_sc", bufs=3, space="PSUM") as pss,
                        tc.tile_pool(name="eps_pv", bufs=2, space="PSUM") as psv,
                        tc.tile_pool(name="eps_sm", bufs=1, space="PSUM") as pssm,
                        tc.tile_pool(name="fps", bufs=2, space="PSUM") as psf,
                  ):
                      # attn stored fp8 hi/lo, (token, head)-interleaved
                      attn_hi = ap.tile([128, QH * S], f8, tag="ath", name="ath")
                      attn_lo = ap.tile([128, QH * S], f8, tag="atl", name="atl")
                      ah_w = attn_hi[:].rearrange("p (s d) -> p s d", d=QH)
                      al_w = attn_lo[:].rearrange("p (s d) -> p s d", d=QH)
                      ah_r = attn_hi[:].rearrange("p (s d) -> p d s", d=QH)
                      al_r = attn_lo[:].rearrange("p (s d) -> p d s", d=QH)

                      def oproj_block(t4):
                        for oc in range(NK):
                            ps_y = psf.tile([128, 512], f32, tag="y", name="y")
                            first = True
                            for ti, (wr, at3) in enumerate(
                                    ((0, ah_r), (1, ah_r), (0, al_r))):
                                for dp in range(2):
                                    off = wr * 8192 + (dp * 16 + oc) * 256
                                    lhs = wo_sb[:, off:off + 256].rearrange(
                                        "p (two m) -> p two m", two=2)
                                    nc.tensor.matmul(
                                        ps_y[:], lhs,
                                        at3[:, 2 * dp:2 * dp + 2,
                                            t4 * 512:(t4 + 1) * 512],
                                        start=first,
                                        stop=(ti == 2 and dp == 1),
                                        perf_mode=DR, skip_group_check=True)
                                    first = False
                            yb = aw.tile([128, 512], bf16, tag="yb", name="yb",
                                         bufs=6)
                            # Pool cannot read PSUM; alternate DVE/ACT
                            if (oc + t4) % 2 == 0:
                                nc.vector.tensor_scalar_mul(yb[:], ps_y[:],
                                                            O_DESCALE)
                            else:
                                nc.scalar.mul(yb[:], ps_y[:], O_DESCALE)
                            nc.sync.dma_start(
                                out=yT_d[oc * 128:(oc + 1) * 128,
                                         t4 * 512:(t4 + 1) * 512],
                                in_=yb[:])

                      for qi in range(NT):
                        for i in range(KVH):
                            qc = qi * 256
                            jstart = max(0, qc // 128 - 8)
                            jend = qc // 128 + 1
                            ps_o = psv.tile([128, 512], f32, tag="pv", name="pv")
                            ps_s = pssm.tile([1, 512], f32, tag="sm", name="sm")
                            q3 = qpair[i][:].rearrange("p (s h) -> p s h", h=2)
                            jorder = list(range(jstart, jend + 1))
                            if qc - jstart * 128 == 1024:
                                # window-edge tile is half-masked; run it
                                # second so a full tile carries the psum clear
                                jorder[0], jorder[1] = jorder[1], jorder[0]
                            jfirst = jorder[0]
                            # in the chain-bound warmup region PE has
                            # slack: inject the mask via an identity matmul
                            # into the scores psum instead of a DVE add
                            pemask = qi <= 1
                            for j in jorder:
                                ps_sc = pss.tile([128, 512], f32, tag="sc",
                                                 name="sc")
                                lhs = kconv[i][:, j * 128:(j + 1) * 128]
                                mt = {1024: 0, 896: 1, 0: 2, -128: 3}.get(qc - j * 128)
                                pb = awe.tile([128, 512], f32r, tag="pb",
                                              name="pb", bufs=6)
                                if j != jfirst and qc - j * 128 == 1024:
                                    sl = slice(0, 256)
                                    qsl = q3[:, qc:qc + 128, :]
                                elif j == jend:
                                    sl = slice(256, 512)
                                    qsl = q3[:, qc + 128:qc + 256, :]
                                else:
                                    sl = slice(0, 512)
                                    qsl = q3[:, qc:qc + 256, :]
                                half = sl != slice(0, 512)
                                if mt is not None and pemask:
                                    ms = slice(mt * 512 + sl.start,
                                               mt * 512 + sl.stop)
                                    nc.tensor.matmul(ps_sc[:, sl], eye_sb,
                                                     msk_sb[:, ms],
                                                     start=True, stop=False,
                                                     skip_group_check=True)
                                    nc.tensor.matmul(ps_sc[:, sl], lhs, qsl,
                                                     start=False, stop=True,
                                                     skip_group_check=True)
                                    nc.scalar.activation(pb[:, sl],
                                                         ps_sc[:, sl], Exp,
                                                         bias=0.0, scale=SCALE)
                                elif mt is not None:
                                    nc.tensor.matmul(ps_sc[:, sl], lhs, qsl,
                                                     start=True, stop=True)
                                    ms = slice(mt * 512 + sl.start,
                                               mt * 512 + sl.stop)
                                    tm = awe.tile([128, 512], f32, tag="tm",
                                                  name="tm")
                                    nc.vector.tensor_add(
                                        tm[:, sl], ps_sc[:, sl],
                                        msk_sb[:, ms])
                                    nc.scalar.activation(pb[:, sl], tm[:, sl],
                                                         Exp, bias=0.0,
                                                         scale=SCALE)
                                else:
                                    nc.tensor.matmul(ps_sc[:, sl], lhs, qsl,
                                                     start=True, stop=True)
                                    nc.scalar.activation(pb[:, sl],
                                                         ps_sc[:, sl], Exp,
                                                         bias=0.0, scale=SCALE)
                                st = (not half) and j == jfirst
                                sp = (j == jend)
                                nc.tensor.matmul(ps_o[:, sl], vt[i][j][:],
                                                 pb[:, sl], start=st, stop=sp)
                                nc.tensor.matmul(ps_s[:, sl], one_sb[:, 0:1],
                                                 pb[:, sl], start=st, stop=sp)
                            # normalize + fp8 hi/lo split: rsum = A_AT/denom
                            # (the ones vector is 1/A_AT), tm = ps_o*rb =
                            # attn*A_AT, hi = fp8(tm) on Pool, lo on DVE
                            rsum = aw.tile([1, 512], f32, tag="rs", name="rs")
                            nc.vector.reciprocal(rsum[:], ps_s[:])
                            rb = aw.tile([128, 512], f32, tag="rb", name="rb")
                            nc.gpsimd.partition_broadcast(rb[:], rsum[:])
                            tmn = aw.tile([128, 512], f32, tag="tmn", name="tmn")
                            tmn3 = tmn.rearrange("p (s h) -> p s h", h=2)
                            ah_sl = ah_w[:, qc:qc + 256, 2 * i:2 * i + 2]
                            po3 = ps_o[:].rearrange("p (s h) -> p s h", h=2)
                            rb3 = rb[:].rearrange("p (s h) -> p s h", h=2)
                            nc.vector.tensor_mul(ah_sl, po3, rb3)
                            nc.vector.tensor_mul(tmn3, po3, rb3)
                            nc.vector.tensor_sub(
                                al_w[:, qc:qc + 256, 2 * i:2 * i + 2], tmn3,
                                ah_sl)
                        if qi % 2 == 1:
                            oproj_block(qi // 2)

    nc.finalize()
    return nc


def _split8(x, scale):
    e4 = ml_dtypes.float8_e4m3
    xs = (x * scale).astype(np.float32)
    hi = xs.astype(e4)
    lo = (xs - hi.astype(np.float32)).astype(e4)
    return hi, lo


def _host_inputs(hidden, W_pack, W_o, conv_k, conv_v):
    """Per-core input maps (fp8 hi/lo staged)."""
    bf = ml_dtypes.bfloat16
    e4 = ml_dtypes.float8_e4m3
    pos = np.arange(S, dtype=np.float64)
    inv_freq = 1.0 / (THETA ** (np.arange(0, HD, 2, dtype=np.float64) / HD))
    freqs = np.outer(pos, inv_freq)                       # (S, 64)
    cos = (np.cos(freqs).T * QKV_DESCALE).astype(np.float32)   # (64, S)
    sin = (np.sin(freqs).T * QKV_DESCALE).astype(np.float32)
    cs = np.concatenate([cos, cos], axis=0)               # (128, S)
    sn = np.concatenate([sin, sin], axis=0)

    kk = np.arange(128)[:, None]
    qq = np.arange(256)[None, :]
    def inter(m):
        # (token, head)-interleaved mask: col 2q+h carries mask[k, q]
        return np.repeat(m, 2, axis=1).astype(np.float32)
    t0 = inter(np.where(kk <= qq, 0.0, NEG))              # delta = 0
    tm128 = inter(np.where(kk <= qq - 128, 0.0, NEG))     # delta = -128
    w896 = inter(np.where(qq - kk < 128, 0.0, NEG))       # delta = 896
    w1024 = inter(np.where(qq < kk, 0.0, NEG))            # delta = 1024
    msk = np.concatenate([w1024, w896, t0, tm128], axis=1)  # (128, 2048)

    eye = np.eye(128, dtype=np.float32)

    csn = np.concatenate([cs, sn], axis=1).astype(bf)     # (128, 4096)
    pre = np.concatenate([cs[:, 0:256], sn[:, 0:256]], axis=1).astype(bf)

    in_maps = []
    for c in range(NCORES):
        b, g = c // TP, c % TP
        hT = np.ascontiguousarray(hidden[b].T)                # (2048, 2048)
        h_hi, h_lo = _split8(hT, A_H)
        wq = W_pack[:, g * 512:(g + 1) * 512]
        wk = W_pack[:, NH * HD + 2 * g * 128: NH * HD + (2 * g + 2) * 128]
        wv = W_pack[:, NH * HD + NKV * HD + 2 * g * 128:
                    NH * HD + NKV * HD + (2 * g + 2) * 128]
        wpk = np.ascontiguousarray(
            np.concatenate([wq, wk, wv], axis=1))             # (2048, 1024)
        w_hi, w_lo = _split8(wpk, B_W)
        cwv = np.empty(8, np.float32)
        for i in range(KVH):
            cwv[2 * i] = conv_k[2 * g + i, 0]
            cwv[2 * i + 1] = conv_k[2 * g + i, 1]
            cwv[4 + 2 * i] = conv_v[2 * g + i, 0] * QKV_DESCALE
            cwv[4 + 2 * i + 1] = conv_v[2 * g + i, 1] * QKV_DESCALE
        cw = np.broadcast_to(cwv, (128, 8)).astype(np.float32).copy()

        # stage0 blob: 8 k-pair groups [wh|xh0|wl|xl0]
        st0 = np.empty((128, _ST0), e4)
        for kp in range(NKP):
            a = kp * _GRP
            k0, k1 = 2 * kp, 2 * kp + 1
            st0[:, a:a + 256] = h_hi[k0 * 128:(k0 + 1) * 128, 0:256]
            st0[:, a + 256:a + 512] = h_hi[k1 * 128:(k1 + 1) * 128, 0:256]
            st0[:, a + 2560:a + 2816] = h_lo[k0 * 128:(k0 + 1) * 128, 0:256]
            st0[:, a + 2816:a + 3072] = h_lo[k1 * 128:(k1 + 1) * 128, 0:256]
            for c in range(8):
                for ki, kblk in ((0, k0), (1, k1)):
                    st0[:, a + 512 + c * 256 + ki * 128:
                        a + 512 + c * 256 + ki * 128 + 128] = \
                        w_hi[kblk * 128:(kblk + 1) * 128,
                             c * 128:(c + 1) * 128]
                    st0[:, a + 3072 + c * 256 + ki * 128:
                        a + 3072 + c * 256 + ki * 128 + 128] = \
                        w_lo[kblk * 128:(kblk + 1) * 128,
                             c * 128:(c + 1) * 128]

        # chunks 1-7 blob: per chunk, per k-pair: [xh(k0)|xh(k1)|xl(k0)|xl(k1)]
        hb = np.empty((128, 7 * 8192), e4)
        for t in range(1, NT):
            base = (t - 1) * 8192
            tok = slice(t * 256, (t + 1) * 256)
            for kp in range(NKP):
                a = base + kp * 1024
                k0, k1 = 2 * kp, 2 * kp + 1
                hb[:, a:a + 256] = h_hi[k0 * 128:(k0 + 1) * 128, tok]
                hb[:, a + 256:a + 512] = h_hi[k1 * 128:(k1 + 1) * 128, tok]
                hb[:, a + 512:a + 768] = h_lo[k0 * 128:(k0 + 1) * 128, tok]
                hb[:, a + 768:a + 1024] = h_lo[k1 * 128:(k1 + 1) * 128, tok]

        # o_proj weights: hi region then lo region; within each, (dp, oc)
        # groups of [w(2dp)[oc-tile] | w(2dp+1)[oc-tile]] (256 cols each)
        wo = W_o[g * 512:(g + 1) * 512, :]                    # (512, 2048)
        wo_hi, wo_lo = _split8(wo, B_WO)
        wob = np.empty((128, 2 * QH * 2048), e4)
        for r, w8 in ((0, wo_hi), (1, wo_lo)):
            for dp in range(2):
                for oc in range(NK):
                    a = r * 8192 + (dp * 16 + oc) * 256
                    d0, d1 = 2 * dp, 2 * dp + 1
                    wob[:, a:a + 128] = w8[d0 * 128:(d0 + 1) * 128,
                                           oc * 128:(oc + 1) * 128]
                    wob[:, a + 128:a + 256] = w8[d1 * 128:(d1 + 1) * 128,
                                                 oc * 128:(oc + 1) * 128]

        # ones column carries 1/A_AT so rsum = A_AT/denom
        oe = np.concatenate([np.full((128, 128), 1.0 / A_AT, np.float32),
                             eye], axis=1)

        in_maps.append({
            "cw": cw, "oe": oe, "pre": pre, "st0": st0, "csn": csn, "hb": hb,
            "wob": wob, "msk": msk,
        })
    return in_maps


def run_cores(in_maps, trace=False, **kw):
    from concourse.bass_utils import run_bass_kernel_spmd
    if "nc" not in _CACHE:
        _CACHE["nc"] = _build_program()
    return run_bass_kernel_spmd(_CACHE["nc"], in_maps, list(range(NCORES)),
                                trace=trace, **kw)


def kernel(hidden, W_pack, W_o, conv_k, conv_v):
    hidden = np.asarray(hidden, np.float32)
    W_pack = np.asarray(W_pack, np.float32)
    W_o = np.asarray(W_o, np.float32)
    conv_k = np.asarray(conv_k, np.float32)
    conv_v = np.asarray(conv_v, np.float32)
    in_maps = _host_inputs(hidden, W_pack, W_o, conv_k, conv_v)
    res = run_cores(in_maps)
    out = np.zeros((B, S, H), np.float32)
    for c in range(NCORES):
        b = c // TP
        out[b] += res.results[c]["yT"].T.astype(np.float32)
    return out


# revision 48
# speedup vs baseline: 1.0113x; 1.0113x over previous
"""Baichuan sliding-window GQA attention block on 8 trn2 NeuronCores.

Sharding: data-parallel over batch (2) x tensor-parallel over heads (4).
Core c handles batch b=c//4, head group g=c%4 (q heads 4g..4g+3, kv heads
2g..2g+1). Each core computes qkv projection, RoPE, 2-tap causal conv,
windowed attention and a row-sharded o_proj partial; the host sums the 4
partials per batch.

All on-chip tensors live in a transposed (feature, token) layout so the
tensor engine's contraction (partition) axis lines up without transposes:
  scoresT[k,q] = sum_d kT[d,k] qT[d,q];  outT[d,q] = sum_k v[k,d] probsT[k,q]
V alone is flipped to (token, dim) via PE transposes.

The qkv projection and o_proj run as fp8 DoubleRow matmuls (2 contraction
tiles per instruction at 0.5 cycles/row) with an error-compensated 3-term
split: x ~ x_hi + x_lo, w ~ w_hi + w_lo (each e4m3, power-of-2 pre-scaled
so the lo residual stays in normal range), and
x@w ~ x_hi@w_hi + x_lo@w_hi + x_hi@w_lo  (0.75x the bf16 cycle count).
The fixed descales fold into the rope cos/sin tables, the v-conv weights,
the softmax ones-vector and the output copy. Attention internals (scores,
probs, V) stay fp32.

Layouts are chosen so the dependency tracker's range checks stay
chunk-local: qpair is (token, head)-interleaved and attn is
(token, head)-interleaved, so attention unit (qi,i) reads only chunk-qi
byte ranges of qpair and o_proj block t4 reads only its own token range
of attn. PSUM pools are flat and shared by tag across phases (qkv chunk
tiles double as attention scores tiles, pv doubles as o_proj psum), so
attention's first allocations chain onto the earliest-freed banks of the
last qkv chunk. o_proj blocks are emitted between attention chunk-pairs
(block t4 right after qi=2*t4+1) so its DoubleRows fill attention's
softmax-latency gaps and the output DMA spreads across the phase.
"""

import numpy as np
import ml_dtypes

B, S, H = 2, 2048, 2048
NH, NKV, HD = 16, 8, 128
WINDOW = 1024
THETA = 100000.0
TP = 4                      # tensor-parallel ways (head groups)
QH = NH // TP               # 4 q heads per core
KVH = NKV // TP             # 2 kv heads per core
NCORES = 8
SCALE = 1.0 / float(np.sqrt(HD))
NEG = -1.0e30

NT = S // 256               # 8 token chunks of 256
NK = H // 128               # 16 contraction tiles
NKP = NK // 2               # 8 contraction tile-pairs (DoubleRow)

# fp8 split scales (powers of two; descale folded into constants)
A_H = 16.0                  # hidden
B_W = 256.0                 # W_pack
QKV_DESCALE = 1.0 / (A_H * B_W)
A_AT = 32.0                 # attn (max |attn| <= max|v| ~ 5.5 -> 176 < 240)
B_WO = 256.0                # W_o
O_DESCALE = 1.0 / (A_AT * B_WO)

# stage0 blob layout (f8e4 cols): 8 k-pair groups of
# [wh_pair 2048 | xh_pair 512 | wl_pair 2048 | xl_pair 512]
_GRP = 5120
_ST0 = NKP * _GRP           # 40960

_CACHE = {}


def _build_program():
    import concourse.bacc as bacc
    import concourse.mybir as mybir
    import concourse.tile as tile

    f32 = mybir.dt.float32
    f32r = mybir.dt.float32r
    bf16 = mybir.dt.bfloat16
    f8 = mybir.dt.float8e4
    DR = mybir.MatmulPerfMode.DoubleRow
    Exp = mybir.ActivationFunctionType.Exp
    mult = mybir.AluOpType.mult
    add = mybir.AluOpType.add

    nc = bacc.Bacc("TRN2", target_bir_lowering=False, debug=False,
                   enable_asserts=False, num_devices=NCORES)

    cw_d = nc.dram_tensor("cw", [128, 8], f32, kind="ExternalInput")
    oe_d = nc.dram_tensor("oe", [128, 256], f32r, kind="ExternalInput")
    pre_d = nc.dram_tensor("pre", [128, 512], bf16, kind="ExternalInput")
    st0_d = nc.dram_tensor("st0", [128, _ST0], f8, kind="ExternalInput")
    csn_d = nc.dram_tensor("csn", [128, 2 * S], bf16, kind="ExternalInput")
    hb_d = nc.dram_tensor("hb", [128, 7 * 8192], f8, kind="ExternalInput")
    wob_d = nc.dram_tensor("wob", [128, 2 * QH * 2048], f8, kind="ExternalInput")
    msk_d = nc.dram_tensor("msk", [128, 2048], f32r, kind="ExternalInput")
    yT_d = nc.dram_tensor("yT", [H, S], bf16, kind="ExternalOutput")

    with tile.TileContext(nc) as tc:
        with (
            tc.tile_pool(name="const", bufs=1) as cp,
            tc.tile_pool(name="persist", bufs=1) as pp,
        ):
            cw_sb = cp.tile([128, 8], f32, tag="cw", name="cw_sb")
            oe_sb = cp.tile([128, 256], f32r, tag="oe", name="oe_sb")
            pre_sb = cp.tile([128, 512], bf16, tag="pre", name="pre_sb")
            wo_sb = cp.tile([128, 2 * QH * 2048], f8, tag="wob", name="wo_sb")
            msk_sb = cp.tile([128, 2048], f32r, tag="msk", name="msk_sb")

            # persistent across phases; qpair is (token, head)-interleaved
            qpair = [pp.tile([128, 2 * S], f32r, tag=f"qp{i}", name=f"qp{i}") for i in range(KVH)]
            kconv = [pp.tile([128, S], f32r, tag=f"kc{i}", name=f"kc{i}") for i in range(KVH)]
            vt = [[pp.tile([128, 128], f32r, tag=f"vt{i}_{j}", name=f"vt{i}_{j}") for j in range(NK)]
                  for i in range(KVH)]

            with tc.tile_pool(name="bst", bufs=1) as bs:
                st0 = bs.tile([128, _ST0], f8, tag="st0", name="st0")
                one_sb = oe_sb[:, 0:128]        # value 1/A_AT
                eye_sb = oe_sb[:, 128:256]

                def whv(kp, c):
                    v = st0[:, kp * _GRP + 512 + c * 256:
                            kp * _GRP + 512 + c * 256 + 256]
                    return v.rearrange("p (two m) -> p two m", two=2)

                def xh0v(kp):
                    v = st0[:, kp * _GRP:kp * _GRP + 512]
                    return v.rearrange("p (two n) -> p two n", two=2)

                def wlv(kp, c):
                    v = st0[:, kp * _GRP + 3072 + c * 256:
                            kp * _GRP + 3072 + c * 256 + 256]
                    return v.rearrange("p (two m) -> p two m", two=2)

                def xl0v(kp):
                    v = st0[:, kp * _GRP + 2560:kp * _GRP + 3072]
                    return v.rearrange("p (two n) -> p two n", two=2)

                # exp-path pool allocated before the phase-B pools so its
                # space is fresh (no WAR on phase-B drains at attention start)
                with tc.tile_pool(name="awE", bufs=3) as awe:
                  with (
                    tc.tile_pool(name="bcsn", bufs=1) as bc,
                    tc.tile_pool(name="bhb", bufs=3) as bh,
                    tc.tile_pool(name="broll", bufs=1) as br,
                    tc.tile_pool(name="btmp", bufs=2) as bt,
                    tc.tile_pool(name="bps", bufs=5, space="PSUM") as psb,
                    tc.tile_pool(name="bps2", bufs=2, space="PSUM") as pse,
                    tc.tile_pool(name="bpst", bufs=1, space="PSUM") as psm,
                  ):
                    # ---- phase B: fused qkv projection + rope + conv +
                    # v-transpose ----
                    for kp in range(NKP):
                        ga = kp * _GRP
                        nsp = 2
                        w = _GRP // nsp
                        for sp in range(nsp):
                            nc.sync.dma_start(
                                out=st0[:, ga + sp * w:ga + (sp + 1) * w],
                                in_=st0_d[:, ga + sp * w:ga + (sp + 1) * w])
                        if kp == 1:
                            nc.sync.dma_start(out=pre_sb[:], in_=pre_d[:, :])
                            nc.sync.dma_start(out=cw_sb[:], in_=cw_d[:, :])
                            nc.sync.dma_start(out=oe_sb[:], in_=oe_d[:, :])
                    csn_sb = bc.tile([128, 2 * S], bf16, tag="csn", name="csn_sb")
                    hbt = []
                    for t in range(1, NT):
                        ht = bh.tile([128, 8192], f8, tag="hb", name=f"hb{t}")
                        hbt.append(ht)
                        nq = 4 if t == 1 else 2
                        for qtr in range(nq):
                            w = 8192 // nq
                            nc.sync.dma_start(
                                out=ht[:, qtr * w:(qtr + 1) * w],
                                in_=hb_d[:, (t - 1) * 8192 + qtr * w:
                                         (t - 1) * 8192 + (qtr + 1) * w])
                        if t == 2:
                            nc.sync.dma_start(out=csn_sb[:], in_=csn_d[:, :])
                            nc.sync.dma_start(out=wo_sb[:], in_=wob_d[:, :])
                            nc.sync.dma_start(out=msk_sb[:], in_=msk_d[:, :])

                    def xhv(t, kp):
                        v = hbt[t - 1][:, kp * 1024:kp * 1024 + 512]
                        return v.rearrange("p (two n) -> p two n", two=2)

                    def xlv(t, kp):
                        v = hbt[t - 1][:, kp * 1024 + 512:kp * 1024 + 1024]
                        return v.rearrange("p (two n) -> p two n", two=2)

                    kbuf = br.tile([128, 1024], f32, name="kbuf")
                    vbuf = br.tile([128, 1024], f32, name="vbuf")
                    for t in range(NT):
                        cur, prv = (t % 2) * 256, ((t + 1) % 2) * 256
                        if t == 0:
                            csl = pre_sb[:, 0:256]
                            snl = pre_sb[:, 256:512]
                        else:
                            csl = csn_sb[:, t * 256:(t + 1) * 256]
                            snl = csn_sb[:, S + t * 256:S + (t + 1) * 256]
                        if t == 0:
                            # kp-outer over all 8 cols (4 psum banks of col
                            # pairs): each arriving DMA piece unlocks a wave
                            # of DoubleRows, so PE ramps with the DMA
                            psc0 = [psb.tile([128, 512], f32, tag="qkps",
                                             name=f"qk0_{c}") for c in range(4)]
                            for kp in range(NKP):
                                for c in range(8):
                                    # start=True lazily zeroes the whole 2KB
                                    # psum bank: only the even half may carry
                                    # it; the odd half's kp=0 lands on bytes
                                    # already marked pending-zero
                                    nc.tensor.matmul(
                                        psc0[c // 2][:, (c % 2) * 256:(c % 2) * 256 + 256],
                                        whv(kp, c),
                                        xh0v(kp),
                                        start=(kp == 0 and c % 2 == 0),
                                        stop=False, perf_mode=DR,
                                        skip_group_check=True)
                                for c in range(8):
                                    nc.tensor.matmul(
                                        psc0[c // 2][:, (c % 2) * 256:(c % 2) * 256 + 256],
                                        wlv(kp, c),
                                        xh0v(kp),
                                        start=False, stop=False, perf_mode=DR,
                                        skip_group_check=True)
                                    nc.tensor.matmul(
                                        psc0[c // 2][:, (c % 2) * 256:(c % 2) * 256 + 256],
                                        whv(kp, c),
                                        xl0v(kp),
                                        start=False,
                                        stop=(kp == NKP - 1), perf_mode=DR,
                                        skip_group_check=True)
                        # emit all 8 cols' DoubleRows first, then drain with
                        # the conv chain emitted right after k/v
                        porder = (1, 0, 3, 2) if t == NT - 1 else (3, 2, 0, 1)
                        pstile = {}
                        for c4 in porder:
                            if t == 0:
                                pstile[c4] = psc0[c4]
                            else:
                                ps2 = psb.tile([128, 512], f32, tag="qkps",
                                               name="qkps")
                                pstile[c4] = ps2
                                for hh in range(2):
                                    col = 2 * c4 + hh
                                    po = ps2[:, hh * 256:(hh + 1) * 256]
                                    for kp in range(NKP):
                                        lhs_h = whv(kp, col)
                                        lhs_l = wlv(kp, col)
                                        nc.tensor.matmul(
                                            po, lhs_h, xhv(t, kp),
                                            start=(kp == 0), stop=False,
                                            perf_mode=DR, skip_group_check=True)
                                        nc.tensor.matmul(
                                            po, lhs_l, xhv(t, kp),
                                            start=False, stop=False,
                                            perf_mode=DR, skip_group_check=True)
                                        nc.tensor.matmul(
                                            po, lhs_h, xlv(t, kp),
                                            start=False, stop=(kp == NKP - 1),
                                            perf_mode=DR, skip_group_check=True)

                        csb = csl.unsqueeze(1).broadcast_to([128, 2, 256])
                        snb = snl.unsqueeze(1).broadcast_to([128, 2, 256])

                        def drain_pair(c4):
                            # both columns of the psum pair drained in single
                            # wide DVE ops: cos/sin broadcast over the column
                            # dim, outputs viewed (s, col) to match the
                            # interleaved destinations
                            ps = pstile[c4][:]
                            if c4 == 3:
                                nc.scalar.copy(vbuf[:, cur * 2:cur * 2 + 512],
                                               ps)
                                return
                            e1 = bt.tile([128, 512], f32, tag="e1", name="e1")
                            # e2 must be PSUM: SB+SB operands with different
                            # base partitions are illegal
                            e2 = pse.tile([128, 512], f32, tag="e2",
                                          name="e2")
                            ps3 = ps.rearrange("p (h s) -> p h s", h=2)
                            nc.vector.tensor_mul(
                                e1[:].rearrange("p (h s) -> p h s", h=2),
                                ps3, csb)
                            nc.vector.tensor_mul(
                                e2[:].rearrange("p (h s) -> p h s", h=2),
                                ps3, snb)
                            e1v = e1[:].rearrange("p (h s) -> p s h", h=2)
                            e2v = e2[:].rearrange("p (h s) -> p s h", h=2)
                            if c4 < 2:
                                q3 = qpair[c4][:].rearrange(
                                    "p (s h) -> p s h", h=2)
                                d0 = q3[0:64, t * 256:(t + 1) * 256, :]
                                d1 = q3[64:128, t * 256:(t + 1) * 256, :]
                            else:
                                k3 = kbuf[:].rearrange(
                                    "p (ki s) -> p s ki", ki=2)
                                d0 = k3[0:64, cur:cur + 256, :]
                                d1 = k3[64:128, cur:cur + 256, :]
                            nc.vector.tensor_sub(d0, e1v[0:64, :, :],
                                                 e2v[64:128, :, :])
                            nc.vector.tensor_add(d1, e2v[0:64, :, :],
                                                 e1v[64:128, :, :])

                        if t == NT - 1:
                            drain_pair(1)
                            drain_pair(0)
                        drain_pair(3)

                        def conv2(src_cur, src_prev1, dst, dst_off, w0c,
                                  w1c, tagp):
                            # 2-tap causal conv on Pool (TensorTensor with
                            # stride-0 broadcast weights; Pool cannot run
                            # TensorScalarPtr or touch PSUM). src_cur is the
                            # chunk's [128,256] slice; src_prev1 the previous
                            # chunk's last column.
                            w0 = cw_sb[:, w0c:w0c + 1]
                            w1 = cw_sb[:, w1c:w1c + 1]
                            tmA = bt.tile([128, 256], f32, tag=tagp + "a",
                                          name=tagp + "a")
                            tmB = bt.tile([128, 256], f32, tag=tagp + "b",
                                          name=tagp + "b")
                            nc.gpsimd.tensor_mul(tmA[:], src_cur,
                                                 w1.broadcast_to([128, 256]))
                            nc.gpsimd.tensor_mul(tmB[:, 1:256],
                                                 src_cur[:, 0:255],
                                                 w0.broadcast_to([128, 255]))
                            if t == 0:
                                nc.gpsimd.tensor_copy(dst[:, dst_off:dst_off + 1],
                                                      tmA[:, 0:1])
                            else:
                                nc.gpsimd.tensor_mul(tmB[:, 0:1], src_prev1, w0)
                                nc.gpsimd.tensor_add(dst[:, dst_off:dst_off + 1],
                                                     tmA[:, 0:1], tmB[:, 0:1])
                            nc.gpsimd.tensor_add(
                                dst[:, dst_off + 1:dst_off + 256],
                                tmA[:, 1:256], tmB[:, 1:256])

                        for i in range(KVH):
                            vcb = bt.tile([128, 256], f32r, tag="vcb", name="vcb")
                            conv2(vbuf[:, cur * 2 + i * 256:
                                       cur * 2 + i * 256 + 256],
                                  vbuf[:, prv * 2 + i * 256 + 255:
                                       prv * 2 + i * 256 + 256],
                                  vcb, 0, 4 + 2 * i, 5 + 2 * i, "cv")
                            for h in range(2):
                                tp = psm.tile([128, 128], f32r, tag="vtp", name="vtp")
                                nc.tensor.transpose(tp[:], vcb[:, h * 128:(h + 1) * 128],
                                                    eye_sb)
                                nc.scalar.copy(vt[i][2 * t + h][:], tp[:])
                        drain_pair(2)
                        for i in range(KVH):
                            conv2(kbuf[:, i * 512 + cur:i * 512 + cur + 256],
                                  kbuf[:, i * 512 + prv + 255:
                                       i * 512 + prv + 256],
                                  kconv[i], t * 256, 2 * i, 2 * i + 1, "ck")
                        if t != NT - 1:
                            drain_pair(0)
                            drain_pair(1)

                  # ---- phase E: banded attention, with o_proj (phase F)
                  # blocks interleaved after every odd chunk ----
                  with (
                        tc.tile_pool(name="att", bufs=1) as ap,
                        tc.tile_pool(name="atw", bufs=3) as aw,
                        tc.tile_pool(name="eps_sc", bufs=3, space="PSUM") as pss,
                        tc.tile_pool(name="eps_pv", bufs=2, space="PSUM") as psv,
                        tc.tile_pool(name="eps_sm", bufs=1, space="PSUM") as pssm,
                        tc.tile_pool(name="fps", bufs=2, space="PSUM") as psf,
                  ):
                      # attn stored fp8 hi/lo, (token, head)-interleaved
                      attn_hi = ap.tile([128, QH * S], f8, tag="ath", name="ath")
                      attn_lo = ap.tile([128, QH * S], f8, tag="atl", name="atl")
                      ah_w = attn_hi[:].rearrange("p (s d) -> p s d", d=QH)
                      al_w = attn_lo[:].rearrange("p (s d) -> p s d", d=QH)
                      ah_r = attn_hi[:].rearrange("p (s d) -> p d s", d=QH)
                      al_r = attn_lo[:].rearrange("p (s d) -> p d s", d=QH)

                      def oproj_block(t4):
                        for oc in range(NK):
                            ps_y = psf.tile([128, 512], f32, tag="y", name="y")
                            first = True
                            for ti, (wr, at3) in enumerate(
                                    ((0, ah_r), (1, ah_r), (0, al_r))):
                                for dp in range(2):
                                    off = wr * 8192 + (dp * 16 + oc) * 256
                                    lhs = wo_sb[:, off:off + 256].rearrange(
                                        "p (two m) -> p two m", two=2)
                                    nc.tensor.matmul(
                                        ps_y[:], lhs,
                                        at3[:, 2 * dp:2 * dp + 2,
                                            t4 * 512:(t4 + 1) * 512],
                                        start=first,
                                        stop=(ti == 2 and dp == 1),
                                        perf_mode=DR, skip_group_check=True)
                                    first = False
                            yb = aw.tile([128, 512], bf16, tag="yb", name="yb",
                                         bufs=6)
                            # Pool cannot read PSUM; alternate DVE/ACT
                            if (oc + t4) % 2 == 0:
                                nc.vector.tensor_scalar_mul(yb[:], ps_y[:],
                                                            O_DESCALE)
                            else:
                                nc.scalar.mul(yb[:], ps_y[:], O_DESCALE)
                            nc.sync.dma_start(
                                out=yT_d[oc * 128:(oc + 1) * 128,
                                         t4 * 512:(t4 + 1) * 512],
                                in_=yb[:])

                      for qi in range(NT):
                        for i in range(KVH):
                            qc = qi * 256
                            jstart = max(0, qc // 128 - 8)
                            jend = qc // 128 + 1
                            ps_o = psv.tile([128, 512], f32, tag="pv", name="pv")
                            ps_s = pssm.tile([1, 512], f32, tag="sm", name="sm")
                            q3 = qpair[i][:].rearrange("p (s h) -> p s h", h=2)
                            jorder = list(range(jstart, jend + 1))
                            if qc - jstart * 128 == 1024:
                                # window-edge tile is half-masked; run it
                                # second so a full tile carries the psum clear
                                jorder[0], jorder[1] = jorder[1], jorder[0]
                            jfirst = jorder[0]
                            # in the chain-bound warmup region PE has
                            # slack: inject the mask via an identity matmul
                            # into the scores psum instead of a DVE add
                            pemask = qi <= 2 or (qi == 3 and i == 0)
                            for j in jorder:
                                ps_sc = pss.tile([128, 512], f32, tag="sc",
                                                 name="sc")
                                lhs = kconv[i][:, j * 128:(j + 1) * 128]
                                mt = {1024: 0, 896: 1, 0: 2, -128: 3}.get(qc - j * 128)
                                pb = awe.tile([128, 512], f32r, tag="pb",
                                              name="pb", bufs=6)
                                if j != jfirst and qc - j * 128 == 1024:
                                    sl = slice(0, 256)
                                    qsl = q3[:, qc:qc + 128, :]
                                elif j == jend:
                                    sl = slice(256, 512)
                                    qsl = q3[:, qc + 128:qc + 256, :]
                                else:
                                    sl = slice(0, 512)
                                    qsl = q3[:, qc:qc + 256, :]
                                half = sl != slice(0, 512)
                                if mt is not None and pemask:
                                    ms = slice(mt * 512 + sl.start,
                                               mt * 512 + sl.stop)
                                    nc.tensor.matmul(ps_sc[:, sl], eye_sb,
                                                     msk_sb[:, ms],
                                                     start=True, stop=False,
                                                     skip_group_check=True)
                                    nc.tensor.matmul(ps_sc[:, sl], lhs, qsl,
                                                     start=False, stop=True,
                                                     skip_group_check=True)
                                    nc.scalar.activation(pb[:, sl],
                                                         ps_sc[:, sl], Exp,
                                                         bias=0.0, scale=SCALE)
                                elif mt is not None:
                                    nc.tensor.matmul(ps_sc[:, sl], lhs, qsl,
                                                     start=True, stop=True)
                                    ms = slice(mt * 512 + sl.start,
                                               mt * 512 + sl.stop)
                                    tm = awe.tile([128, 512], f32, tag="tm",
                                                  name="tm")
                                    nc.vector.tensor_add(
                                        tm[:, sl], ps_sc[:, sl],
                                        msk_sb[:, ms])
                                    nc.scalar.activation(pb[:, sl], tm[:, sl],
                                                         Exp, bias=0.0,
                                                         scale=SCALE)
                                else:
                                    nc.tensor.matmul(ps_sc[:, sl], lhs, qsl,
                                                     start=True, stop=True)
                                    nc.scalar.activation(pb[:, sl],
                                                         ps_sc[:, sl], Exp,
                                                         bias=0.0, scale=SCALE)
                                st = (not half) and j == jfirst
                                sp = (j == jend)
                                nc.tensor.matmul(ps_o[:, sl], vt[i][j][:],
                                                 pb[:, sl], start=st, stop=sp)
                                nc.tensor.matmul(ps_s[:, sl], one_sb[:, 0:1],
                                                 pb[:, sl], start=st, stop=sp)
                            # normalize + fp8 hi/lo split: rsum = A_AT/denom
                            # (the ones vector is 1/A_AT), tm = ps_o*rb =
                            # attn*A_AT, hi = fp8(tm) on Pool, lo on DVE
                            rsum = aw.tile([1, 512], f32, tag="rs", name="rs")
                            nc.vector.reciprocal(rsum[:], ps_s[:])
                            rb = aw.tile([128, 512], f32, tag="rb", name="rb")
                            nc.gpsimd.partition_broadcast(rb[:], rsum[:])
                            tmn = aw.tile([128, 512], f32, tag="tmn", name="tmn")
                            tmn3 = tmn.rearrange("p (s h) -> p s h", h=2)
                            ah_sl = ah_w[:, qc:qc + 256, 2 * i:2 * i + 2]
                            po3 = ps_o[:].rearrange("p (s h) -> p s h", h=2)
                            rb3 = rb[:].rearrange("p (s h) -> p s h", h=2)
                            nc.vector.tensor_mul(ah_sl, po3, rb3)
                            nc.vector.tensor_mul(tmn3, po3, rb3)
                            nc.vector.tensor_sub(
                                al_w[:, qc:qc + 256, 2 * i:2 * i + 2], tmn3,
                                ah_sl)
                        if qi % 2 == 1:
                            oproj_block(qi // 2)

    nc.finalize()
    return nc


def _split8(x, scale):
    e4 = ml_dtypes.float8_e4m3
    xs = (x * scale).astype(np.float32)
    hi = xs.astype(e4)
    lo = (xs - hi.astype(np.float32)).astype(e4)
    return hi, lo


def _host_inputs(hidden, W_pack, W_o, conv_k, conv_v):
    """Per-core input maps (fp8 hi/lo staged)."""
    bf = ml_dtypes.bfloat16
    e4 = ml_dtypes.float8_e4m3
    pos = np.arange(S, dtype=np.float64)
    inv_freq = 1.0 / (THETA ** (np.arange(0, HD, 2, dtype=np.float64) / HD))
    freqs = np.outer(pos, inv_freq)                       # (S, 64)
    cos = (np.cos(freqs).T * QKV_DESCALE).astype(np.float32)   # (64, S)
    sin = (np.sin(freqs).T * QKV_DESCALE).astype(np.float32)
    cs = np.concatenate([cos, cos], axis=0)               # (128, S)
    sn = np.concatenate([sin, sin], axis=0)

    kk = np.arange(128)[:, None]
    qq = np.arange(256)[None, :]
    def inter(m):
        # (token, head)-interleaved mask: col 2q+h carries mask[k, q]
        return np.repeat(m, 2, axis=1).astype(np.float32)
    t0 = inter(np.where(kk <= qq, 0.0, NEG))              # delta = 0
    tm128 = inter(np.where(kk <= qq - 128, 0.0, NEG))     # delta = -128
    w896 = inter(np.where(qq - kk < 128, 0.0, NEG))       # delta = 896
    w1024 = inter(np.where(qq < kk, 0.0, NEG))            # delta = 1024
    msk = np.concatenate([w1024, w896, t0, tm128], axis=1)  # (128, 2048)

    eye = np.eye(128, dtype=np.float32)

    csn = np.concatenate([cs, sn], axis=1).astype(bf)     # (128, 4096)
    pre = np.concatenate([cs[:, 0:256], sn[:, 0:256]], axis=1).astype(bf)

    in_maps = []
    for c in range(NCORES):
        b, g = c // TP, c % TP
        hT = np.ascontiguousarray(hidden[b].T)                # (2048, 2048)
        h_hi, h_lo = _split8(hT, A_H)
        wq = W_pack[:, g * 512:(g + 1) * 512]
        wk = W_pack[:, NH * HD + 2 * g * 128: NH * HD + (2 * g + 2) * 128]
        wv = W_pack[:, NH * HD + NKV * HD + 2 * g * 128:
                    NH * HD + NKV * HD + (2 * g + 2) * 128]
        wpk = np.ascontiguousarray(
            np.concatenate([wq, wk, wv], axis=1))             # (2048, 1024)
        w_hi, w_lo = _split8(wpk, B_W)
        cwv = np.empty(8, np.float32)
        for i in range(KVH):
            cwv[2 * i] = conv_k[2 * g + i, 0]
            cwv[2 * i + 1] = conv_k[2 * g + i, 1]
            cwv[4 + 2 * i] = conv_v[2 * g + i, 0] * QKV_DESCALE
            cwv[4 + 2 * i + 1] = conv_v[2 * g + i, 1] * QKV_DESCALE
        cw = np.broadcast_to(cwv, (128, 8)).astype(np.float32).copy()

        # stage0 blob: 8 k-pair groups [wh|xh0|wl|xl0]
        st0 = np.empty((128, _ST0), e4)
        for kp in range(NKP):
            a = kp * _GRP
            k0, k1 = 2 * kp, 2 * kp + 1
            st0[:, a:a + 256] = h_hi[k0 * 128:(k0 + 1) * 128, 0:256]
            st0[:, a + 256:a + 512] = h_hi[k1 * 128:(k1 + 1) * 128, 0:256]
            st0[:, a + 2560:a + 2816] = h_lo[k0 * 128:(k0 + 1) * 128, 0:256]
            st0[:, a + 2816:a + 3072] = h_lo[k1 * 128:(k1 + 1) * 128, 0:256]
            for c in range(8):
                for ki, kblk in ((0, k0), (1, k1)):
                    st0[:, a + 512 + c * 256 + ki * 128:
                        a + 512 + c * 256 + ki * 128 + 128] = \
                        w_hi[kblk * 128:(kblk + 1) * 128,
                             c * 128:(c + 1) * 128]
                    st0[:, a + 3072 + c * 256 + ki * 128:
                        a + 3072 + c * 256 + ki * 128 + 128] = \
                        w_lo[kblk * 128:(kblk + 1) * 128,
                             c * 128:(c + 1) * 128]

        # chunks 1-7 blob: per chunk, per k-pair: [xh(k0)|xh(k1)|xl(k0)|xl(k1)]
        hb = np.empty((128, 7 * 8192), e4)
        for t in range(1, NT):
            base = (t - 1) * 8192
            tok = slice(t * 256, (t + 1) * 256)
            for kp in range(NKP):
                a = base + kp * 1024
                k0, k1 = 2 * kp, 2 * kp + 1
                hb[:, a:a + 256] = h_hi[k0 * 128:(k0 + 1) * 128, tok]
                hb[:, a + 256:a + 512] = h_hi[k1 * 128:(k1 + 1) * 128, tok]
                hb[:, a + 512:a + 768] = h_lo[k0 * 128:(k0 + 1) * 128, tok]
                hb[:, a + 768:a + 1024] = h_lo[k1 * 128:(k1 + 1) * 128, tok]

        # o_proj weights: hi region then lo region; within each, (dp, oc)
        # groups of [w(2dp)[oc-tile] | w(2dp+1)[oc-tile]] (256 cols each)
        wo = W_o[g * 512:(g + 1) * 512, :]                    # (512, 2048)
        wo_hi, wo_lo = _split8(wo, B_WO)
        wob = np.empty((128, 2 * QH * 2048), e4)
        for r, w8 in ((0, wo_hi), (1, wo_lo)):
            for dp in range(2):
                for oc in range(NK):
                    a = r * 8192 + (dp * 16 + oc) * 256
                    d0, d1 = 2 * dp, 2 * dp + 1
                    wob[:, a:a + 128] = w8[d0 * 128:(d0 + 1) * 128,
                                           oc * 128:(oc + 1) * 128]
                    wob[:, a + 128:a + 256] = w8[d1 * 128:(d1 + 1) * 128,
                                                 oc * 128:(oc + 1) * 128]

        # ones column carries 1/A_AT so rsum = A_AT/denom
        oe = np.concatenate([np.full((128, 128), 1.0 / A_AT, np.float32),
                             eye], axis=1)

        in_maps.append({
            "cw": cw, "oe": oe, "pre": pre, "st0": st0, "csn": csn, "hb": hb,
            "wob": wob, "msk": msk,
        })
    return in_maps


def run_cores(in_maps, trace=False, **kw):
    from concourse.bass_utils import run_bass_kernel_spmd
    if "nc" not in _CACHE:
        _CACHE["nc"] = _build_program()
    return run_bass_kernel_spmd(_CACHE["nc"], in_maps, list(range(NCORES)),
                                trace=trace, **kw)


def kernel(hidden, W_pack, W_o, conv_k, conv_v):
    hidden = np.asarray(hidden, np.float32)
    W_pack = np.asarray(W_pack, np.float32)
    W_o = np.asarray(W_o, np.float32)
    conv_k = np.asarray(conv_k, np.float32)
    conv_v = np.asarray(conv_v, np.float32)
    in_maps = _host_inputs(hidden, W_pack, W_o, conv_k, conv_v)
    res = run_cores(in_maps)
    out = np.zeros((B, S, H), np.float32)
    for c in range(NCORES):
        b = c // TP
        out[b] += res.results[c]["yT"].T.astype(np.float32)
    return out


# revision 49
# speedup vs baseline: 1.0121x; 1.0008x over previous
"""Baichuan sliding-window GQA attention block on 8 trn2 NeuronCores.

Sharding: data-parallel over batch (2) x tensor-parallel over heads (4).
Core c handles batch b=c//4, head group g=c%4 (q heads 4g..4g+3, kv heads
2g..2g+1). Each core computes qkv projection, RoPE, 2-tap causal conv,
windowed attention and a row-sharded o_proj partial; the host sums the 4
partials per batch.

All on-chip tensors live in a transposed (feature, token) layout so the
tensor engine's contraction (partition) axis lines up without transposes:
  scoresT[k,q] = sum_d kT[d,k] qT[d,q];  outT[d,q] = sum_k v[k,d] probsT[k,q]
V alone is flipped to (token, dim) via PE transposes.

The qkv projection and o_proj run as fp8 DoubleRow matmuls (2 contraction
tiles per instruction at 0.5 cycles/row) with an error-compensated 3-term
split: x ~ x_hi + x_lo, w ~ w_hi + w_lo (each e4m3, power-of-2 pre-scaled
so the lo residual stays in normal range), and
x@w ~ x_hi@w_hi + x_lo@w_hi + x_hi@w_lo  (0.75x the bf16 cycle count).
The fixed descales fold into the rope cos/sin tables, the v-conv weights,
the softmax ones-vector and the output copy. Attention internals (scores,
probs, V) stay fp32.

Layouts are chosen so the dependency tracker's range checks stay
chunk-local: qpair is (token, head)-interleaved and attn is
(token, head)-interleaved, so attention unit (qi,i) reads only chunk-qi
byte ranges of qpair and o_proj block t4 reads only its own token range
of attn. PSUM pools are flat and shared by tag across phases (qkv chunk
tiles double as attention scores tiles, pv doubles as o_proj psum), so
attention's first allocations chain onto the earliest-freed banks of the
last qkv chunk. o_proj blocks are emitted between attention chunk-pairs
(block t4 right after qi=2*t4+1) so its DoubleRows fill attention's
softmax-latency gaps and the output DMA spreads across the phase.
"""

import numpy as np
import ml_dtypes

B, S, H = 2, 2048, 2048
NH, NKV, HD = 16, 8, 128
WINDOW = 1024
THETA = 100000.0
TP = 4                      # tensor-parallel ways (head groups)
QH = NH // TP               # 4 q heads per core
KVH = NKV // TP             # 2 kv heads per core
NCORES = 8
SCALE = 1.0 / float(np.sqrt(HD))
NEG = -1.0e30

NT = S // 256               # 8 token chunks of 256
NK = H // 128               # 16 contraction tiles
NKP = NK // 2               # 8 contraction tile-pairs (DoubleRow)

# fp8 split scales (powers of two; descale folded into constants)
A_H = 16.0                  # hidden
B_W = 256.0                 # W_pack
QKV_DESCALE = 1.0 / (A_H * B_W)
A_AT = 32.0                 # attn (max |attn| <= max|v| ~ 5.5 -> 176 < 240)
B_WO = 256.0                # W_o
O_DESCALE = 1.0 / (A_AT * B_WO)

# stage0 blob layout (f8e4 cols): 8 k-pair groups of
# [wh_pair 2048 | xh_pair 512 | wl_pair 2048 | xl_pair 512]
_GRP = 5120
_ST0 = NKP * _GRP           # 40960

_CACHE = {}


def _build_program():
    import concourse.bacc as bacc
    import concourse.mybir as mybir
    import concourse.tile as tile

    f32 = mybir.dt.float32
    f32r = mybir.dt.float32r
    bf16 = mybir.dt.bfloat16
    f8 = mybir.dt.float8e4
    DR = mybir.MatmulPerfMode.DoubleRow
    Exp = mybir.ActivationFunctionType.Exp
    mult = mybir.AluOpType.mult
    add = mybir.AluOpType.add

    nc = bacc.Bacc("TRN2", target_bir_lowering=False, debug=False,
                   enable_asserts=False, num_devices=NCORES)

    cw_d = nc.dram_tensor("cw", [128, 8], f32, kind="ExternalInput")
    oe_d = nc.dram_tensor("oe", [128, 256], f32r, kind="ExternalInput")
    pre_d = nc.dram_tensor("pre", [128, 512], bf16, kind="ExternalInput")
    st0_d = nc.dram_tensor("st0", [128, _ST0], f8, kind="ExternalInput")
    csn_d = nc.dram_tensor("csn", [128, 2 * S], bf16, kind="ExternalInput")
    hb_d = nc.dram_tensor("hb", [128, 7 * 8192], f8, kind="ExternalInput")
    wob_d = nc.dram_tensor("wob", [128, 2 * QH * 2048], f8, kind="ExternalInput")
    msk_d = nc.dram_tensor("msk", [128, 2048], f32r, kind="ExternalInput")
    yT_d = nc.dram_tensor("yT", [H, S], bf16, kind="ExternalOutput")

    with tile.TileContext(nc) as tc:
        with (
            tc.tile_pool(name="const", bufs=1) as cp,
            tc.tile_pool(name="persist", bufs=1) as pp,
        ):
            cw_sb = cp.tile([128, 8], f32, tag="cw", name="cw_sb")
            oe_sb = cp.tile([128, 256], f32r, tag="oe", name="oe_sb")
            pre_sb = cp.tile([128, 512], bf16, tag="pre", name="pre_sb")
            wo_sb = cp.tile([128, 2 * QH * 2048], f8, tag="wob", name="wo_sb")
            msk_sb = cp.tile([128, 2048], f32r, tag="msk", name="msk_sb")

            # persistent across phases; qpair is (token, head)-interleaved
            qpair = [pp.tile([128, 2 * S], f32r, tag=f"qp{i}", name=f"qp{i}") for i in range(KVH)]
            kconv = [pp.tile([128, S], f32r, tag=f"kc{i}", name=f"kc{i}") for i in range(KVH)]
            vt = [[pp.tile([128, 128], f32r, tag=f"vt{i}_{j}", name=f"vt{i}_{j}") for j in range(NK)]
                  for i in range(KVH)]

            with tc.tile_pool(name="bst", bufs=1) as bs:
                st0 = bs.tile([128, _ST0], f8, tag="st0", name="st0")
                one_sb = oe_sb[:, 0:128]        # value 1/A_AT
                eye_sb = oe_sb[:, 128:256]

                def whv(kp, c):
                    v = st0[:, kp * _GRP + 512 + c * 256:
                            kp * _GRP + 512 + c * 256 + 256]
                    return v.rearrange("p (two m) -> p two m", two=2)

                def xh0v(kp):
                    v = st0[:, kp * _GRP:kp * _GRP + 512]
                    return v.rearrange("p (two n) -> p two n", two=2)

                def wlv(kp, c):
                    v = st0[:, kp * _GRP + 3072 + c * 256:
                            kp * _GRP + 3072 + c * 256 + 256]
                    return v.rearrange("p (two m) -> p two m", two=2)

                def xl0v(kp):
                    v = st0[:, kp * _GRP + 2560:kp * _GRP + 3072]
                    return v.rearrange("p (two n) -> p two n", two=2)

                # exp-path pool allocated before the phase-B pools so its
                # space is fresh (no WAR on phase-B drains at attention start)
                with tc.tile_pool(name="awE", bufs=3) as awe:
                  with (
                    tc.tile_pool(name="bcsn", bufs=1) as bc,
                    tc.tile_pool(name="bhb", bufs=3) as bh,
                    tc.tile_pool(name="broll", bufs=1) as br,
                    tc.tile_pool(name="btmp", bufs=2) as bt,
                    tc.tile_pool(name="bps", bufs=5, space="PSUM") as psb,
                    tc.tile_pool(name="bps2", bufs=2, space="PSUM") as pse,
                    tc.tile_pool(name="bpst", bufs=1, space="PSUM") as psm,
                  ):
                    # ---- phase B: fused qkv projection + rope + conv +
                    # v-transpose ----
                    for kp in range(NKP):
                        ga = kp * _GRP
                        nsp = 2
                        w = _GRP // nsp
                        for sp in range(nsp):
                            nc.sync.dma_start(
                                out=st0[:, ga + sp * w:ga + (sp + 1) * w],
                                in_=st0_d[:, ga + sp * w:ga + (sp + 1) * w])
                        if kp == 1:
                            nc.sync.dma_start(out=pre_sb[:], in_=pre_d[:, :])
                            nc.sync.dma_start(out=cw_sb[:], in_=cw_d[:, :])
                            nc.sync.dma_start(out=oe_sb[:], in_=oe_d[:, :])
                    csn_sb = bc.tile([128, 2 * S], bf16, tag="csn", name="csn_sb")
                    hbt = []
                    for t in range(1, NT):
                        ht = bh.tile([128, 8192], f8, tag="hb", name=f"hb{t}")
                        hbt.append(ht)
                        nq = 4 if t == 1 else 2
                        for qtr in range(nq):
                            w = 8192 // nq
                            nc.sync.dma_start(
                                out=ht[:, qtr * w:(qtr + 1) * w],
                                in_=hb_d[:, (t - 1) * 8192 + qtr * w:
                                         (t - 1) * 8192 + (qtr + 1) * w])
                        if t == 2:
                            nc.sync.dma_start(out=csn_sb[:], in_=csn_d[:, :])
                            nc.sync.dma_start(out=wo_sb[:], in_=wob_d[:, :])
                            nc.sync.dma_start(out=msk_sb[:], in_=msk_d[:, :])

                    def xhv(t, kp):
                        v = hbt[t - 1][:, kp * 1024:kp * 1024 + 512]
                        return v.rearrange("p (two n) -> p two n", two=2)

                    def xlv(t, kp):
                        v = hbt[t - 1][:, kp * 1024 + 512:kp * 1024 + 1024]
                        return v.rearrange("p (two n) -> p two n", two=2)

                    kbuf = br.tile([128, 1024], f32, name="kbuf")
                    vbuf = br.tile([128, 1024], f32, name="vbuf")
                    for t in range(NT):
                        cur, prv = (t % 2) * 256, ((t + 1) % 2) * 256
                        if t == 0:
                            csl = pre_sb[:, 0:256]
                            snl = pre_sb[:, 256:512]
                        else:
                            csl = csn_sb[:, t * 256:(t + 1) * 256]
                            snl = csn_sb[:, S + t * 256:S + (t + 1) * 256]
                        if t == 0:
                            # kp-outer over all 8 cols (4 psum banks of col
                            # pairs): each arriving DMA piece unlocks a wave
                            # of DoubleRows, so PE ramps with the DMA
                            psc0 = [psb.tile([128, 512], f32, tag="qkps",
                                             name=f"qk0_{c}") for c in range(4)]
                            for kp in range(NKP):
                                for c in range(8):
                                    # start=True lazily zeroes the whole 2KB
                                    # psum bank: only the even half may carry
                                    # it; the odd half's kp=0 lands on bytes
                                    # already marked pending-zero
                                    nc.tensor.matmul(
                                        psc0[c // 2][:, (c % 2) * 256:(c % 2) * 256 + 256],
                                        whv(kp, c),
                                        xh0v(kp),
                                        start=(kp == 0 and c % 2 == 0),
                                        stop=False, perf_mode=DR,
                                        skip_group_check=True)
                                for c in range(8):
                                    nc.tensor.matmul(
                                        psc0[c // 2][:, (c % 2) * 256:(c % 2) * 256 + 256],
                                        wlv(kp, c),
                                        xh0v(kp),
                                        start=False, stop=False, perf_mode=DR,
                                        skip_group_check=True)
                                    nc.tensor.matmul(
                                        psc0[c // 2][:, (c % 2) * 256:(c % 2) * 256 + 256],
                                        whv(kp, c),
                                        xl0v(kp),
                                        start=False,
                                        stop=(kp == NKP - 1), perf_mode=DR,
                                        skip_group_check=True)
                        # emit all 8 cols' DoubleRows first, then drain with
                        # the conv chain emitted right after k/v
                        porder = (1, 0, 3, 2) if t == NT - 1 else (3, 2, 0, 1)
                        pstile = {}
                        for c4 in porder:
                            if t == 0:
                                pstile[c4] = psc0[c4]
                            else:
                                ps2 = psb.tile([128, 512], f32, tag="qkps",
                                               name="qkps")
                                pstile[c4] = ps2
                                for hh in range(2):
                                    col = 2 * c4 + hh
                                    po = ps2[:, hh * 256:(hh + 1) * 256]
                                    for kp in range(NKP):
                                        lhs_h = whv(kp, col)
                                        lhs_l = wlv(kp, col)
                                        nc.tensor.matmul(
                                            po, lhs_h, xhv(t, kp),
                                            start=(kp == 0), stop=False,
                                            perf_mode=DR, skip_group_check=True)
                                        nc.tensor.matmul(
                                            po, lhs_l, xhv(t, kp),
                                            start=False, stop=False,
                                            perf_mode=DR, skip_group_check=True)
                                        nc.tensor.matmul(
                                            po, lhs_h, xlv(t, kp),
                                            start=False, stop=(kp == NKP - 1),
                                            perf_mode=DR, skip_group_check=True)

                        csb = csl.unsqueeze(1).broadcast_to([128, 2, 256])
                        snb = snl.unsqueeze(1).broadcast_to([128, 2, 256])

                        def drain_pair(c4):
                            # both columns of the psum pair drained in single
                            # wide DVE ops: cos/sin broadcast over the column
                            # dim, outputs viewed (s, col) to match the
                            # interleaved destinations
                            ps = pstile[c4][:]
                            if c4 == 3:
                                nc.scalar.copy(vbuf[:, cur * 2:cur * 2 + 512],
                                               ps)
                                return
                            e1 = bt.tile([128, 512], f32, tag="e1", name="e1")
                            # e2 must be PSUM: SB+SB operands with different
                            # base partitions are illegal
                            e2 = pse.tile([128, 512], f32, tag="e2",
                                          name="e2")
                            ps3 = ps.rearrange("p (h s) -> p h s", h=2)
                            nc.vector.tensor_mul(
                                e1[:].rearrange("p (h s) -> p h s", h=2),
                                ps3, csb)
                            nc.vector.tensor_mul(
                                e2[:].rearrange("p (h s) -> p h s", h=2),
                                ps3, snb)
                            e1v = e1[:].rearrange("p (h s) -> p s h", h=2)
                            e2v = e2[:].rearrange("p (h s) -> p s h", h=2)
                            if c4 < 2:
                                q3 = qpair[c4][:].rearrange(
                                    "p (s h) -> p s h", h=2)
                                d0 = q3[0:64, t * 256:(t + 1) * 256, :]
                                d1 = q3[64:128, t * 256:(t + 1) * 256, :]
                            else:
                                k3 = kbuf[:].rearrange(
                                    "p (ki s) -> p s ki", ki=2)
                                d0 = k3[0:64, cur:cur + 256, :]
                                d1 = k3[64:128, cur:cur + 256, :]
                            nc.vector.tensor_sub(d0, e1v[0:64, :, :],
                                                 e2v[64:128, :, :])
                            nc.vector.tensor_add(d1, e2v[0:64, :, :],
                                                 e1v[64:128, :, :])

                        if t == NT - 1:
                            drain_pair(1)
                            drain_pair(0)
                        drain_pair(3)

                        def conv2(src_cur, src_prev1, dst, dst_off, w0c,
                                  w1c, tagp):
                            # 2-tap causal conv on Pool (TensorTensor with
                            # stride-0 broadcast weights; Pool cannot run
                            # TensorScalarPtr or touch PSUM). src_cur is the
                            # chunk's [128,256] slice; src_prev1 the previous
                            # chunk's last column.
                            w0 = cw_sb[:, w0c:w0c + 1]
                            w1 = cw_sb[:, w1c:w1c + 1]
                            tmA = bt.tile([128, 256], f32, tag=tagp + "a",
                                          name=tagp + "a")
                            tmB = bt.tile([128, 256], f32, tag=tagp + "b",
                                          name=tagp + "b")
                            nc.gpsimd.tensor_mul(tmA[:], src_cur,
                                                 w1.broadcast_to([128, 256]))
                            nc.gpsimd.tensor_mul(tmB[:, 1:256],
                                                 src_cur[:, 0:255],
                                                 w0.broadcast_to([128, 255]))
                            if t == 0:
                                nc.gpsimd.tensor_copy(dst[:, dst_off:dst_off + 1],
                                                      tmA[:, 0:1])
                            else:
                                nc.gpsimd.tensor_mul(tmB[:, 0:1], src_prev1, w0)
                                nc.gpsimd.tensor_add(dst[:, dst_off:dst_off + 1],
                                                     tmA[:, 0:1], tmB[:, 0:1])
                            nc.gpsimd.tensor_add(
                                dst[:, dst_off + 1:dst_off + 256],
                                tmA[:, 1:256], tmB[:, 1:256])

                        for i in range(KVH):
                            vcb = bt.tile([128, 256], f32r, tag="vcb", name="vcb")
                            conv2(vbuf[:, cur * 2 + i * 256:
                                       cur * 2 + i * 256 + 256],
                                  vbuf[:, prv * 2 + i * 256 + 255:
                                       prv * 2 + i * 256 + 256],
                                  vcb, 0, 4 + 2 * i, 5 + 2 * i, "cv")
                            for h in range(2):
                                tp = psm.tile([128, 128], f32r, tag="vtp", name="vtp")
                                nc.tensor.transpose(tp[:], vcb[:, h * 128:(h + 1) * 128],
                                                    eye_sb)
                                nc.scalar.copy(vt[i][2 * t + h][:], tp[:])
                        drain_pair(2)
                        for i in range(KVH):
                            conv2(kbuf[:, i * 512 + cur:i * 512 + cur + 256],
                                  kbuf[:, i * 512 + prv + 255:
                                       i * 512 + prv + 256],
                                  kconv[i], t * 256, 2 * i, 2 * i + 1, "ck")
                        if t != NT - 1:
                            drain_pair(0)
                            drain_pair(1)

                  # ---- phase E: banded attention, with o_proj (phase F)
                  # blocks interleaved after every odd chunk ----
                  with (
                        tc.tile_pool(name="att", bufs=1) as ap,
                        tc.tile_pool(name="atw", bufs=3) as aw,
                        tc.tile_pool(name="eps_sc", bufs=3, space="PSUM") as pss,
                        tc.tile_pool(name="eps_pv", bufs=2, space="PSUM") as psv,
                        tc.tile_pool(name="eps_sm", bufs=1, space="PSUM") as pssm,
                        tc.tile_pool(name="fps", bufs=2, space="PSUM") as psf,
                  ):
                      # attn stored fp8 hi/lo, (token, head)-interleaved
                      attn_hi = ap.tile([128, QH * S], f8, tag="ath", name="ath")
                      attn_lo = ap.tile([128, QH * S], f8, tag="atl", name="atl")
                      ah_w = attn_hi[:].rearrange("p (s d) -> p s d", d=QH)
                      al_w = attn_lo[:].rearrange("p (s d) -> p s d", d=QH)
                      ah_r = attn_hi[:].rearrange("p (s d) -> p d s", d=QH)
                      al_r = attn_lo[:].rearrange("p (s d) -> p d s", d=QH)

                      def oproj_block(t4):
                        for oc in range(NK):
                            ps_y = psf.tile([128, 512], f32, tag="y", name="y")
                            first = True
                            for ti, (wr, at3) in enumerate(
                                    ((0, ah_r), (1, ah_r), (0, al_r))):
                                for dp in range(2):
                                    off = wr * 8192 + (dp * 16 + oc) * 256
                                    lhs = wo_sb[:, off:off + 256].rearrange(
                                        "p (two m) -> p two m", two=2)
                                    nc.tensor.matmul(
                                        ps_y[:], lhs,
                                        at3[:, 2 * dp:2 * dp + 2,
                                            t4 * 512:(t4 + 1) * 512],
                                        start=first,
                                        stop=(ti == 2 and dp == 1),
                                        perf_mode=DR, skip_group_check=True)
                                    first = False
                            yb = aw.tile([128, 512], bf16, tag="yb", name="yb",
                                         bufs=6)
                            # Pool cannot read PSUM; alternate DVE/ACT
                            if (oc + t4) % 2 == 0:
                                nc.vector.tensor_scalar_mul(yb[:], ps_y[:],
                                                            O_DESCALE)
                            else:
                                nc.scalar.mul(yb[:], ps_y[:], O_DESCALE)
                            nc.sync.dma_start(
                                out=yT_d[oc * 128:(oc + 1) * 128,
                                         t4 * 512:(t4 + 1) * 512],
                                in_=yb[:])

                      for qi in range(NT):
                        for i in range(KVH):
                            qc = qi * 256
                            jstart = max(0, qc // 128 - 8)
                            jend = qc // 128 + 1
                            ps_o = psv.tile([128, 512], f32, tag="pv", name="pv")
                            ps_s = pssm.tile([1, 512], f32, tag="sm", name="sm")
                            q3 = qpair[i][:].rearrange("p (s h) -> p s h", h=2)
                            jorder = list(range(jstart, jend + 1))
                            if qc - jstart * 128 == 1024:
                                # window-edge tile is half-masked; run it
                                # second so a full tile carries the psum clear
                                jorder[0], jorder[1] = jorder[1], jorder[0]
                            jfirst = jorder[0]
                            # in the chain-bound warmup region PE has
                            # slack: inject the mask via an identity matmul
                            # into the scores psum instead of a DVE add
                            pemask = qi <= 2 or (qi == 3 and i == 0) or (qi == 4 and i == 0)
                            for j in jorder:
                                ps_sc = pss.tile([128, 512], f32, tag="sc",
                                                 name="sc")
                                lhs = kconv[i][:, j * 128:(j + 1) * 128]
                                mt = {1024: 0, 896: 1, 0: 2, -128: 3}.get(qc - j * 128)
                                pb = awe.tile([128, 512], f32r, tag="pb",
                                              name="pb", bufs=6)
                                if j != jfirst and qc - j * 128 == 1024:
                                    sl = slice(0, 256)
                                    qsl = q3[:, qc:qc + 128, :]
                                elif j == jend:
                                    sl = slice(256, 512)
                                    qsl = q3[:, qc + 128:qc + 256, :]
                                else:
                                    sl = slice(0, 512)
                                    qsl = q3[:, qc:qc + 256, :]
                                half = sl != slice(0, 512)
                                if mt is not None and pemask:
                                    ms = slice(mt * 512 + sl.start,
                                               mt * 512 + sl.stop)
                                    nc.tensor.matmul(ps_sc[:, sl], eye_sb,
                                                     msk_sb[:, ms],
                                                     start=True, stop=False,
                                                     skip_group_check=True)
                                    nc.tensor.matmul(ps_sc[:, sl], lhs, qsl,
                                                     start=False, stop=True,
                                                     skip_group_check=True)
                                    nc.scalar.activation(pb[:, sl],
                                                         ps_sc[:, sl], Exp,
                                                         bias=0.0, scale=SCALE)
                                elif mt is not None:
                                    nc.tensor.matmul(ps_sc[:, sl], lhs, qsl,
                                                     start=True, stop=True)
                                    ms = slice(mt * 512 + sl.start,
                                               mt * 512 + sl.stop)
                                    tm = awe.tile([128, 512], f32, tag="tm",
                                                  name="tm")
                                    nc.vector.tensor_add(
                                        tm[:, sl], ps_sc[:, sl],
                                        msk_sb[:, ms])
                                    nc.scalar.activation(pb[:, sl], tm[:, sl],
                                                         Exp, bias=0.0,
                                                         scale=SCALE)
                                else:
                                    nc.tensor.matmul(ps_sc[:, sl], lhs, qsl,
                                                     start=True, stop=True)
                                    nc.scalar.activation(pb[:, sl],
                                                         ps_sc[:, sl], Exp,
                                                         bias=0.0, scale=SCALE)
                                st = (not half) and j == jfirst
                                sp = (j == jend)
                                nc.tensor.matmul(ps_o[:, sl], vt[i][j][:],
                                                 pb[:, sl], start=st, stop=sp)
                                nc.tensor.matmul(ps_s[:, sl], one_sb[:, 0:1],
                                                 pb[:, sl], start=st, stop=sp)
                            # normalize + fp8 hi/lo split: rsum = A_AT/denom
                            # (the ones vector is 1/A_AT), tm = ps_o*rb =
                            # attn*A_AT, hi = fp8(tm) on Pool, lo on DVE
                            rsum = aw.tile([1, 512], f32, tag="rs", name="rs")
                            nc.vector.reciprocal(rsum[:], ps_s[:])
                            rb = aw.tile([128, 512], f32, tag="rb", name="rb")
                            nc.gpsimd.partition_broadcast(rb[:], rsum[:])
                            tmn = aw.tile([128, 512], f32, tag="tmn", name="tmn")
                            tmn3 = tmn.rearrange("p (s h) -> p s h", h=2)
                            ah_sl = ah_w[:, qc:qc + 256, 2 * i:2 * i + 2]
                            po3 = ps_o[:].rearrange("p (s h) -> p s h", h=2)
                            rb3 = rb[:].rearrange("p (s h) -> p s h", h=2)
                            nc.vector.tensor_mul(ah_sl, po3, rb3)
                            nc.vector.tensor_mul(tmn3, po3, rb3)
                            nc.vector.tensor_sub(
                                al_w[:, qc:qc + 256, 2 * i:2 * i + 2], tmn3,
                                ah_sl)
                        if qi % 2 == 1:
                            oproj_block(qi // 2)

    nc.finalize()
    return nc


def _split8(x, scale):
    e4 = ml_dtypes.float8_e4m3
    xs = (x * scale).astype(np.float32)
    hi = xs.astype(e4)
    lo = (xs - hi.astype(np.float32)).astype(e4)
    return hi, lo


def _host_inputs(hidden, W_pack, W_o, conv_k, conv_v):
    """Per-core input maps (fp8 hi/lo staged)."""
    bf = ml_dtypes.bfloat16
    e4 = ml_dtypes.float8_e4m3
    pos = np.arange(S, dtype=np.float64)
    inv_freq = 1.0 / (THETA ** (np.arange(0, HD, 2, dtype=np.float64) / HD))
    freqs = np.outer(pos, inv_freq)                       # (S, 64)
    cos = (np.cos(freqs).T * QKV_DESCALE).astype(np.float32)   # (64, S)
    sin = (np.sin(freqs).T * QKV_DESCALE).astype(np.float32)
    cs = np.concatenate([cos, cos], axis=0)               # (128, S)
    sn = np.concatenate([sin, sin], axis=0)

    kk = np.arange(128)[:, None]
    qq = np.arange(256)[None, :]
    def inter(m):
        # (token, head)-interleaved mask: col 2q+h carries mask[k, q]
        return np.repeat(m, 2, axis=1).astype(np.float32)
    t0 = inter(np.where(kk <= qq, 0.0, NEG))              # delta = 0
    tm128 = inter(np.where(kk <= qq - 128, 0.0, NEG))     # delta = -128
    w896 = inter(np.where(qq - kk < 128, 0.0, NEG))       # delta = 896
    w1024 = inter(np.where(qq < kk, 0.0, NEG))            # delta = 1024
    msk = np.concatenate([w1024, w896, t0, tm128], axis=1)  # (128, 2048)

    eye = np.eye(128, dtype=np.float32)

    csn = np.concatenate([cs, sn], axis=1).astype(bf)     # (128, 4096)
    pre = np.concatenate([cs[:, 0:256], sn[:, 0:256]], axis=1).astype(bf)

    in_maps = []
    for c in range(NCORES):
        b, g = c // TP, c % TP
        hT = np.ascontiguousarray(hidden[b].T)                # (2048, 2048)
        h_hi, h_lo = _split8(hT, A_H)
        wq = W_pack[:, g * 512:(g + 1) * 512]
        wk = W_pack[:, NH * HD + 2 * g * 128: NH * HD + (2 * g + 2) * 128]
        wv = W_pack[:, NH * HD + NKV * HD + 2 * g * 128:
                    NH * HD + NKV * HD + (2 * g + 2) * 128]
        wpk = np.ascontiguousarray(
            np.concatenate([wq, wk, wv], axis=1))             # (2048, 1024)
        w_hi, w_lo = _split8(wpk, B_W)
        cwv = np.empty(8, np.float32)
        for i in range(KVH):
            cwv[2 * i] = conv_k[2 * g + i, 0]
            cwv[2 * i + 1] = conv_k[2 * g + i, 1]
            cwv[4 + 2 * i] = conv_v[2 * g + i, 0] * QKV_DESCALE
            cwv[4 + 2 * i + 1] = conv_v[2 * g + i, 1] * QKV_DESCALE
        cw = np.broadcast_to(cwv, (128, 8)).astype(np.float32).copy()

        # stage0 blob: 8 k-pair groups [wh|xh0|wl|xl0]
        st0 = np.empty((128, _ST0), e4)
        for kp in range(NKP):
            a = kp * _GRP
            k0, k1 = 2 * kp, 2 * kp + 1
            st0[:, a:a + 256] = h_hi[k0 * 128:(k0 + 1) * 128, 0:256]
            st0[:, a + 256:a + 512] = h_hi[k1 * 128:(k1 + 1) * 128, 0:256]
            st0[:, a + 2560:a + 2816] = h_lo[k0 * 128:(k0 + 1) * 128, 0:256]
            st0[:, a + 2816:a + 3072] = h_lo[k1 * 128:(k1 + 1) * 128, 0:256]
            for c in range(8):
                for ki, kblk in ((0, k0), (1, k1)):
                    st0[:, a + 512 + c * 256 + ki * 128:
                        a + 512 + c * 256 + ki * 128 + 128] = \
                        w_hi[kblk * 128:(kblk + 1) * 128,
                             c * 128:(c + 1) * 128]
                    st0[:, a + 3072 + c * 256 + ki * 128:
                        a + 3072 + c * 256 + ki * 128 + 128] = \
                        w_lo[kblk * 128:(kblk + 1) * 128,
                             c * 128:(c + 1) * 128]

        # chunks 1-7 blob: per chunk, per k-pair: [xh(k0)|xh(k1)|xl(k0)|xl(k1)]
        hb = np.empty((128, 7 * 8192), e4)
        for t in range(1, NT):
            base = (t - 1) * 8192
            tok = slice(t * 256, (t + 1) * 256)
            for kp in range(NKP):
                a = base + kp * 1024
                k0, k1 = 2 * kp, 2 * kp + 1
                hb[:, a:a + 256] = h_hi[k0 * 128:(k0 + 1) * 128, tok]
                hb[:, a + 256:a + 512] = h_hi[k1 * 128:(k1 + 1) * 128, tok]
                hb[:, a + 512:a + 768] = h_lo[k0 * 128:(k0 + 1) * 128, tok]
                hb[:, a + 768:a + 1024] = h_lo[k1 * 128:(k1 + 1) * 128, tok]

        # o_proj weights: hi region then lo region; within each, (dp, oc)
        # groups of [w(2dp)[oc-tile] | w(2dp+1)[oc-tile]] (256 cols each)
        wo = W_o[g * 512:(g + 1) * 512, :]                    # (512, 2048)
        wo_hi, wo_lo = _split8(wo, B_WO)
        wob = np.empty((128, 2 * QH * 2048), e4)
        for r, w8 in ((0, wo_hi), (1, wo_lo)):
            for dp in range(2):
                for oc in range(NK):
                    a = r * 8192 + (dp * 16 + oc) * 256
                    d0, d1 = 2 * dp, 2 * dp + 1
                    wob[:, a:a + 128] = w8[d0 * 128:(d0 + 1) * 128,
                                           oc * 128:(oc + 1) * 128]
                    wob[:, a + 128:a + 256] = w8[d1 * 128:(d1 + 1) * 128,
                                                 oc * 128:(oc + 1) * 128]

        # ones column carries 1/A_AT so rsum = A_AT/denom
        oe = np.concatenate([np.full((128, 128), 1.0 / A_AT, np.float32),
                             eye], axis=1)

        in_maps.append({
            "cw": cw, "oe": oe, "pre": pre, "st0": st0, "csn": csn, "hb": hb,
            "wob": wob, "msk": msk,
        })
    return in_maps


def run_cores(in_maps, trace=False, **kw):
    from concourse.bass_utils import run_bass_kernel_spmd
    if "nc" not in _CACHE:
        _CACHE["nc"] = _build_program()
    return run_bass_kernel_spmd(_CACHE["nc"], in_maps, list(range(NCORES)),
                                trace=trace, **kw)


def kernel(hidden, W_pack, W_o, conv_k, conv_v):
    hidden = np.asarray(hidden, np.float32)
    W_pack = np.asarray(W_pack, np.float32)
    W_o = np.asarray(W_o, np.float32)
    conv_k = np.asarray(conv_k, np.float32)
    conv_v = np.asarray(conv_v, np.float32)
    in_maps = _host_inputs(hidden, W_pack, W_o, conv_k, conv_v)
    res = run_cores(in_maps)
    out = np.zeros((B, S, H), np.float32)
    for c in range(NCORES):
        b = c // TP
        out[b] += res.results[c]["yT"].T.astype(np.float32)
    return out
